# revision 1
# baseline (speedup 1.0000x reference)
"""HGRN attention Trainium2 kernel (v3, bf16 data path).

Sharding: B*L (4 batches x 4096 tokens) split into 8 chunks of T=2048 tokens,
one per NeuronCore: core c = 2*b + half handles tokens [half*T, (half+1)*T) of
batch b. The gated linear recurrence h_t = sigmoid(f_t)*h_{t-1} + swiglu-input
runs per (batch, channel); the cross-chunk carry (h at the half boundary) is
exchanged with a tiny pairwise AllReduce and applied as h_local + cumprod*carry
(cumprod underflows to exactly 0 in fp32 past ~130 steps, so only the first 256
columns of each odd chunk need the fixup).

All-bf16 data path (rel err ~4e-3 vs the 2e-2 gate). Phase 1 streams x per
512-token quarter (one contiguous DMA from a host-prepacked quarter-major
layout), loops output tiles et inside, and keeps the gated output
p = g*gnw*silu(h) entirely in SBUF (8 MiB bf16) - no DRAM spills. PSUM is
double-buffered so the PE never waits on consumers. The per-token rmsnorm
scale for quarter q is computed DURING quarter q+1 (ones-matmul partition
reduce + reciprocal_approx_fast + Sqrt(scale=D)) so the phase transition has
no serial rms chain. Phase 2 holds all of Wo resident bf16: the leading 4
output blocks (wo_a) live in a pool that survives the phase transition and
prefetch during the last quarter - SBUF pool reuse otherwise makes the Wo
load inherit a WAR dependency on the last phase-1 consumers, stalling the PE
past the ~3.4us HAM re-throttle window. Time blocks run in order (1,2,3,0)
so the carry AllReduce hides under ~165us of o_proj compute; the
collective-result DMA is emitted just before block 0 to avoid head-of-line
blocking the DMA issue queue. Measured: ~911us HW exec (baseline 1234us),
PE warm-clocked end-to-end, 3.5us of total PE gaps (2.9us transition +
0.6us startup), matmul issue at the 216ns hardware floor elsewhere.
"""
import numpy as np
import ml_dtypes

import concourse.bacc as bacc
import concourse.tile as tile
import concourse.mybir as mybir
from concourse.bass_utils import run_bass_kernel_spmd

B, L, D = 4, 4096, 2048
T = 2048                 # tokens per core
NCORE = 8
ET = DT = D // 128       # 16 tiles of 128 channels
TQ = 512                 # time block (quarter of T)
NQ = T // TQ
CLEN = 256               # cumprod fixup length (0 in fp32 beyond this)

F32 = mybir.dt.float32
BF16 = mybir.dt.bfloat16
AF = mybir.ActivationFunctionType
OP = mybir.AluOpType

_CACHE = {}


def _build():
    nc = bacc.Bacc("TRN2", target_bir_lowering=False, debug=False,
                   enable_asserts=True, num_devices=NCORE)
    # x in host-prepacked quarter-major layout: row q*128+p, col dt*TQ+t
    xt_d = nc.dram_tensor("xt", [NQ * 128, DT * TQ], BF16,
                          kind="ExternalInput")
    wi_d = nc.dram_tensor("wi", [ET * 128, DT * 128], BF16, kind="ExternalInput")
    wf_d = nc.dram_tensor("wf", [ET * 128, DT * 128], BF16, kind="ExternalInput")
    wg_d = nc.dram_tensor("wg", [ET * 128, DT * 128], BF16, kind="ExternalInput")
    wo_d = nc.dram_tensor("wo", [DT * 128, ET * 128], BF16, kind="ExternalInput")
    gnw_d = nc.dram_tensor("gnw", [128, ET], F32, kind="ExternalInput")
    mask_d = nc.dram_tensor("mask", [128, 1], F32, kind="ExternalInput")
    yt_d = nc.dram_tensor("yt", [D, T], BF16, kind="ExternalOutput")

    with tile.TileContext(nc) as tc:
        with tc.tile_pool(name="persist", bufs=1) as pp, \
             tc.tile_pool(name="dram", bufs=1, space="DRAM") as dr:
            carry = pp.tile([128, ET], F32, tag="carry")
            recv = pp.tile([128, ET], F32, tag="recv")
            cin = pp.tile([128, ET], F32, tag="cin")
            gnw = pp.tile([128, ET], F32, tag="gnw")
            maskt = pp.tile([128, 1], F32, tag="mask")
            rms = pp.tile([128, T], BF16, tag="rms")
            call = pp.tile([128, ET * CLEN], BF16, tag="call")
            hkeep = pp.tile([128, ET * CLEN], BF16, tag="hkeep")
            gkeep = pp.tile([128, ET * CLEN], BF16, tag="gkeep")
            ones = pp.tile([128, 128], BF16, tag="ones")
            psb = pp.tile([128, ET * T], BF16, tag="psb")

            hl_i = dr.tile([128, ET], F32, tag="hli")
            hl_o = dr.tile([128, ET], F32, tag="hlo")

            nc.vector.memset(carry[:], 0.0)
            nc.vector.memset(ones[:], 1.0)

            with tc.tile_pool(name="accp", bufs=1) as ap_, \
                 tc.tile_pool(name="rwk", bufs=1) as rwk, \
                 tc.tile_pool(name="woa", bufs=1) as woap, \
                 tc.tile_pool(name="sp", bufs=1, space="PSUM") as sp:
                acc = ap_.tile([128, T], F32, tag="acc")
                NWA = 4  # leading o_proj weight blocks resident before phase 2
                wo_a = woap.tile([128, NWA * ET * 128], BF16, tag="woa")

                def rms_chunk(tb):
                    # rms[:, tb] = rsqrt(mean(g^2)) = sqrt(D * (1/sum(g^2)))
                    # (EPS=1e-5 vs mean ~0.8 is negligible at bf16 precision;
                    # bf16 partition-reduce keeps the ones-matmul at 1 cyc/row)
                    ts = tb * TQ
                    accb = rwk.tile([128, TQ], BF16, tag="accb")
                    nc.scalar.copy(accb[:], acc[:, ts:ts + TQ])
                    S = sp.tile([128, TQ], F32, tag="S")
                    nc.tensor.matmul(S[:], ones[:], accb[:],
                                     start=True, stop=True)
                    rec = rwk.tile([128, TQ], F32, tag="rec")
                    nc.vector.reciprocal_approx_fast(rec[:], S[:])
                    nc.scalar.activation(rms[:, ts:ts + TQ], rec[:], AF.Sqrt,
                                         scale=float(D))

                # ---------- phase 1: projections + scan, p kept in SBUF -----
                with tc.tile_pool(name="xq", bufs=2) as xqp, \
                     tc.tile_pool(name="wp", bufs=2) as wp, \
                     tc.tile_pool(name="wk", bufs=2) as wk, \
                     tc.tile_pool(name="pj", bufs=2, space="PSUM") as pj:
                    for q in range(NQ):
                        # weights for the first et land before x on q==0 so
                        # the PE starts as early as possible
                        pre_ws = None
                        if q == 0:
                            pre_ws = {}
                            w = wp.tile([128, DT * 128], BF16, tag="wf")
                            nc.sync.dma_start(w[:], wf_d.ap()[0:128, :])
                            pre_ws["wf"] = w
                        xq = xqp.tile([128, DT * TQ], BF16, tag="xq")
                        if q == 0:
                            # 8 chunks so the first matmuls can start while
                            # the rest of the quarter streams in
                            for k in range(8):
                                cw = 2 * TQ
                                nc.sync.dma_start(
                                    xq[:, k * cw:(k + 1) * cw],
                                    xt_d.ap()[0:128, k * cw:(k + 1) * cw])
                            for nm, wd in (("wi", wi_d), ("wg", wg_d)):
                                w = wp.tile([128, DT * 128], BF16, tag=nm)
                                nc.sync.dma_start(w[:], wd.ap()[0:128, :])
                                pre_ws[nm] = w
                            nc.sync.dma_start(gnw[:], gnw_d.ap()[:])
                            nc.sync.dma_start(maskt[:], mask_d.ap()[:])
                        else:
                            nc.sync.dma_start(
                                xq[:], xt_d.ap()[q * 128:(q + 1) * 128, :])
                        if q == NQ - 1:
                            # prefetch the leading o_proj weights so phase 2
                            # starts with zero weight-wait
                            for dt in range(NWA):
                                nc.sync.dma_start(
                                    wo_a[:, dt * ET * 128:(dt + 1) * ET * 128],
                                    wo_d.ap()[dt * 128:(dt + 1) * 128, :])
                        ts0 = q * TQ
                        nc.vector.memset(acc[:, ts0:ts0 + TQ], 0.0)
                        for et in range(ET):
                            if q == 0 and et == 0:
                                ws = pre_ws
                            else:
                                ws = {}
                                for nm, wd in (("wf", wf_d), ("wi", wi_d),
                                               ("wg", wg_d)):
                                    w = wp.tile([128, DT * 128], BF16, tag=nm)
                                    nc.sync.dma_start(
                                        w[:],
                                        wd.ap()[et * 128:(et + 1) * 128, :])
                                    ws[nm] = w
                            ps = {}
                            for nm in ("pf", "pi", "pg"):
                                p = pj.tile([128, TQ], F32, tag=nm)
                                w = ws["w" + nm[1]]
                                for dt in range(DT):
                                    nc.tensor.matmul(
                                        p[:], w[:, dt * 128:(dt + 1) * 128],
                                        xq[:, dt * TQ:(dt + 1) * TQ],
                                        start=(dt == 0), stop=(dt == DT - 1))
                                ps[nm] = p
                            if et == 0 and q >= 1:
                                # acc for quarter q-1 finalized during our
                                # et=0 matmuls; reduce it now (PE in-order:
                                # emitting later avoids stalling this quarter)
                                rms_chunk(q - 1)
                            gate = wk.tile([128, TQ], F32, tag="gate")
                            nc.scalar.activation(gate[:], ps["pf"][:],
                                                 AF.Sigmoid)
                            sil = wk.tile([128, TQ], F32, tag="sil")
                            nc.scalar.activation(sil[:], ps["pi"][:],
                                                 AF.Sigmoid)
                            nc.vector.tensor_mul(sil[:], ps["pi"][:], sil[:])
                            sq = wk.tile([128, TQ], F32, tag="sq")
                            nc.scalar.activation(sq[:], ps["pg"][:], AF.Square)
                            omg = wk.tile([128, TQ], F32, tag="omg")
                            nc.vector.tensor_scalar(omg[:], gate[:], -1.0, 1.0,
                                                    OP.mult, OP.add)
                            iin = wk.tile([128, TQ], F32, tag="iin")
                            nc.vector.tensor_mul(iin[:], omg[:], sil[:])
                            h1 = wk.tile([128, TQ], F32, tag="h1")
                            nc.vector.tensor_tensor_scan(
                                h1[:], gate[:], iin[:], carry[:, et:et + 1],
                                OP.mult, OP.add)
                            nc.vector.tensor_copy(carry[:, et:et + 1],
                                                  h1[:, TQ - 1:TQ])
                            if q == 0:
                                nc.vector.tensor_tensor_scan(
                                    call[:, et * CLEN:(et + 1) * CLEN],
                                    gate[:, 0:CLEN], gate[:, 0:CLEN], 1.0,
                                    OP.mult, OP.bypass)
                                nc.scalar.copy(
                                    hkeep[:, et * CLEN:(et + 1) * CLEN],
                                    h1[:, 0:CLEN])
                                nc.scalar.copy(
                                    gkeep[:, et * CLEN:(et + 1) * CLEN],
                                    ps["pg"][:, 0:CLEN])
                            nc.vector.tensor_add(acc[:, ts0:ts0 + TQ],
                                                 acc[:, ts0:ts0 + TQ], sq[:])
                            sw = wk.tile([128, TQ], F32, tag="sw")
                            nc.scalar.activation(sw[:], h1[:], AF.Sigmoid)
                            nc.vector.tensor_mul(sw[:], h1[:], sw[:])
                            nc.vector.scalar_tensor_tensor(
                                psb[:, et * T + ts0:et * T + ts0 + TQ],
                                ps["pg"][:], gnw[:, et:et + 1], sw[:],
                                OP.mult, OP.mult)

                # ---------- phase 1.5: carry exchange + last rms chunk ------
                nc.sync.dma_start(hl_i[:], carry[:])
                nc.gpsimd.collective_compute(
                    "AllReduce", OP.add,
                    replica_groups=[[0, 1], [2, 3], [4, 5], [6, 7]],
                    ins=[hl_i.opt()], outs=[hl_o.opt()])
                rms_chunk(3)

                # ---------- phase 2: gating + output projection -----------------
                with tc.tile_pool(name="wop", bufs=1) as wop, \
                     tc.tile_pool(name="osp", bufs=2) as osp, \
                     tc.tile_pool(name="fxp", bufs=1) as fxp, \
                     tc.tile_pool(name="yp", bufs=4, space="PSUM") as yp, \
                     tc.tile_pool(name="yo", bufs=4) as yo:
                    wo = wop.tile([128, (DT - NWA) * ET * 128], BF16,
                                  tag="wo")
                    for dt in range(NWA, DT):
                        nc.sync.dma_start(
                            wo[:, (dt - NWA) * ET * 128:
                               (dt - NWA + 1) * ET * 128],
                            wo_d.ap()[dt * 128:(dt + 1) * 128, :])
                    for idx, tb2 in enumerate((1, 2, 3, 0)):
                        ts = tb2 * TQ
                        if tb2 == 0:
                            # collective finished ~150us ago; fetch it now so
                            # this DMA never head-of-line blocks the queue
                            nc.sync.dma_start(recv[:], hl_o[:])
                            nc.vector.tensor_sub(recv[:], recv[:], carry[:])
                            nc.vector.tensor_scalar(cin[:], recv[:],
                                                    maskt[:, 0:1], None,
                                                    OP.mult)
                            for et in range(ET):
                                cs = slice(et * CLEN, (et + 1) * CLEN)
                                hfx = fxp.tile([128, CLEN], F32, tag="hfx")
                                nc.vector.scalar_tensor_tensor(
                                    hfx[:], call[:, cs], cin[:, et:et + 1],
                                    hkeep[:, cs], OP.mult, OP.add)
                                swf = fxp.tile([128, CLEN], F32, tag="swf")
                                nc.scalar.activation(swf[:], hfx[:],
                                                     AF.Sigmoid)
                                nc.vector.tensor_mul(swf[:], hfx[:], swf[:])
                                nc.vector.scalar_tensor_tensor(
                                    psb[:, et * T:et * T + CLEN],
                                    gkeep[:, cs], gnw[:, et:et + 1], swf[:],
                                    OP.mult, OP.mult)
                        ot = osp.tile([128, ET * TQ], BF16, tag="osb")
                        for et in range(ET):
                            nc.vector.tensor_mul(
                                ot[:, et * TQ:(et + 1) * TQ],
                                psb[:, et * T + ts:et * T + ts + TQ],
                                rms[:, ts:ts + TQ])
                        for dt in range(DT):
                            if dt < NWA:
                                wsrc, base = wo_a, dt * ET
                            else:
                                wsrc, base = wo, (dt - NWA) * ET
                            ypt = yp.tile([128, TQ], F32, tag="ypt")
                            for et in range(ET):
                                nc.tensor.matmul(
                                    ypt[:],
                                    wsrc[:, (base + et) * 128:
                                       (base + et + 1) * 128],
                                    ot[:, et * TQ:(et + 1) * TQ],
                                    start=(et == 0), stop=(et == ET - 1))
                            ysb = yo.tile([128, TQ], BF16, tag="ysb")
                            nc.scalar.copy(ysb[:], ypt[:])
                            nc.sync.dma_start(
                                yt_d.ap()[dt * 128:(dt + 1) * 128,
                                          ts:ts + TQ], ysb[:])
    nc.compile()
    return nc


def _get_nc():
    if "nc" not in _CACHE:
        _CACHE["nc"] = _build()
    return _CACHE["nc"]


def _make_in_maps(hidden_states, Wi, Wf, Wg, g_norm_weight, Wo, **_unused):
    bf = ml_dtypes.bfloat16

    def prep_ifg(W):
        # SBUF tile for output block et: w[p, dt*128+e] = W.T[dt*128+p, et*128+e]
        WT = np.ascontiguousarray(np.asarray(W, np.float32).T)
        A = (WT.reshape(DT, 128, ET, 128).transpose(2, 1, 0, 3)
             .reshape(ET * 128, DT * 128))
        return np.ascontiguousarray(A.astype(bf))

    def prep_o(W):
        # SBUF wo[p, (dt*ET+et)*128+d] = W.T[et*128+p, dt*128+d]
        WT = np.ascontiguousarray(np.asarray(W, np.float32).T)
        C = (WT.reshape(ET, 128, DT, 128).transpose(2, 1, 0, 3)
             .reshape(DT * 128, ET * 128))
        return np.ascontiguousarray(C.astype(bf))

    wi = prep_ifg(Wi)
    wf = prep_ifg(Wf)
    wg = prep_ifg(Wg)
    wo = prep_o(Wo)
    gnw = np.ascontiguousarray(
        np.asarray(g_norm_weight, np.float32).reshape(ET, 128).T)
    hs = np.asarray(hidden_states, np.float32)
    in_maps = []
    for c in range(NCORE):
        b, half = c // 2, c % 2
        # quarter-major pack: xt[q*128+p, dt*TQ+t] = x[b, half*T + q*TQ+t,
        # dt*128+p] so each quarter is one contiguous [128, DT*TQ] DMA
        blk = hs[b, half * T:(half + 1) * T, :]
        xt = np.ascontiguousarray(
            blk.reshape(NQ, TQ, DT, 128).transpose(0, 3, 2, 1)
            .reshape(NQ * 128, DT * TQ)).astype(bf)
        mask = np.full((128, 1), float(half), np.float32)
        in_maps.append({"xt": xt, "wi": wi, "wf": wf, "wg": wg,
                        "wo": wo, "gnw": gnw, "mask": mask})
    return in_maps


def kernel(hidden_states, Wi, Wf, Wg, g_norm_weight, Wo, **_unused):
    nc = _get_nc()
    in_maps = _make_in_maps(hidden_states, Wi, Wf, Wg, g_norm_weight, Wo)
    _CACHE["in_maps"] = in_maps
    res = run_bass_kernel_spmd(nc, in_maps, list(range(NCORE))).results
    y = np.empty((B, L, D), np.float32)
    for c in range(NCORE):
        b, half = c // 2, c % 2
        y[b, half * T:(half + 1) * T, :] = res[c]["yt"].T.astype(np.float32)
    return y



# revision 13
# speedup vs baseline: 1.1245x; 1.1245x over previous
"""HGRN attention Trainium2 kernel (v5, bf16 + fp8-f data path).

Sharding: B*L (4 batches x 4096 tokens) split into 8 chunks of T=2048 tokens,
one per NeuronCore: core c = 2*b + half handles tokens [half*T, (half+1)*T) of
batch b. The gated linear recurrence h_t = sigmoid(f_t)*h_{t-1} + swiglu-input
runs per (batch, channel); the cross-chunk carry (h at the half boundary) is
exchanged with a tiny pairwise AllReduce and applied as h_local + cumprod*carry
(cumprod underflows to exactly 0 in fp32 past ~130 steps, so only the first 256
columns of each odd chunk need the fixup).

v5: the f projection (gate input) runs in fp8 e4m3 with perf_mode=DoubleRow
(2 fp8 weights per PE cell contract 256 channels per instruction, ~1.8x the
bf16 rate); all other GEMMs stay bf16. Per-GEMM fp8 error decomposition
(sim_decomp.py): e_f=1.4e-2 (sigmoid-attenuated), e_i/e_g=3.1e-2,
e_o=3.7e-2 on the 2e-2 budget - only f fits. Operands are quantized
host-side with per-tensor absmax scales; the descale rides the Sigmoid's
ACT scale operand as a runtime [128,1] SBUF scalar, so one compile serves
any input scaling.

Otherwise the v3 structure (rel err ~4e-3 bf16-only). Phase 1 streams x per
512-token quarter (one contiguous DMA from a host-prepacked quarter-major
layout), loops output tiles et inside, and keeps the gated output
p = g*gnw*silu(h) entirely in SBUF (8 MiB bf16) - no DRAM spills. PSUM is
double-buffered so the PE never waits on consumers. The per-token rmsnorm
scale for quarter q is computed DURING quarter q+1 (ones-matmul partition
reduce + reciprocal_approx_fast + Sqrt(scale=D)) so the phase transition has
no serial rms chain. Phase 2 holds all of Wo resident bf16: the leading 4
output blocks (wo_a) live in a pool that survives the phase transition and
prefetch during the last quarter - SBUF pool reuse otherwise makes the Wo
load inherit a WAR dependency on the last phase-1 consumers, stalling the PE
past the ~3.4us HAM re-throttle window. Time blocks run in order (1,2,3,0)
so the carry AllReduce hides under ~165us of o_proj compute; the
collective-result DMA is emitted just before block 0 to avoid head-of-line
blocking the DMA issue queue. Measured: ~911us HW exec (baseline 1234us),
PE warm-clocked end-to-end, 3.5us of total PE gaps (2.9us transition +
0.6us startup), matmul issue at the 216ns hardware floor elsewhere.
"""
import numpy as np
import ml_dtypes

import concourse.bacc as bacc
import concourse.tile as tile
import concourse.mybir as mybir
from concourse.bass_utils import run_bass_kernel_spmd

B, L, D = 4, 4096, 2048
T = 2048                 # tokens per core
NCORE = 8
ET = DT = D // 128       # 16 tiles of 128 channels
TQ = 512                 # time block (quarter of T)
NQ = T // TQ
CLEN = 256               # cumprod fixup length (0 in fp32 beyond this)

F32 = mybir.dt.float32
BF16 = mybir.dt.bfloat16
F8 = mybir.dt.float8e4
AF = mybir.ActivationFunctionType
OP = mybir.AluOpType
DR = mybir.MatmulPerfMode.DoubleRow

_CACHE = {}


def _build():
    nc = bacc.Bacc("TRN2", target_bir_lowering=False, debug=False,
                   enable_asserts=True, num_devices=NCORE)
    # x in host-prepacked quarter-major layout: row q*128+p, col dt*TQ+t
    xt_d = nc.dram_tensor("xt", [NQ * 128, DT * TQ], BF16,
                          kind="ExternalInput")
    xt8_d = nc.dram_tensor("xt8", [NQ * 128, DT, TQ], F8,
                           kind="ExternalInput")
    wi_d = nc.dram_tensor("wi", [ET * 128, DT * 128], BF16, kind="ExternalInput")
    wf_d = nc.dram_tensor("wf", [ET * 128, DT, 128], F8, kind="ExternalInput")
    wg_d = nc.dram_tensor("wg", [ET * 128, DT * 128], BF16, kind="ExternalInput")
    wo_d = nc.dram_tensor("wo", [DT * 128, ET * 128], BF16, kind="ExternalInput")
    gnw_d = nc.dram_tensor("gnw", [128, ET], F32, kind="ExternalInput")
    mask_d = nc.dram_tensor("mask", [128, 1], F32, kind="ExternalInput")
    # runtime descale for the fp8 f projection: df = 1/(sx*swf)
    dfs_d = nc.dram_tensor("dfs", [128, 1], F32, kind="ExternalInput")
    yt_d = nc.dram_tensor("yt", [D, T], BF16, kind="ExternalOutput")

    with tile.TileContext(nc) as tc:
        with tc.tile_pool(name="persist", bufs=1) as pp, \
             tc.tile_pool(name="dram", bufs=1, space="DRAM") as dr:
            carry = pp.tile([128, ET], F32, tag="carry")
            recv = pp.tile([128, ET], F32, tag="recv")
            cin = pp.tile([128, ET], F32, tag="cin")
            gnw = pp.tile([128, ET], F32, tag="gnw")
            maskt = pp.tile([128, 1], F32, tag="mask")
            dfs = pp.tile([128, 1], F32, tag="dfs")
            rms = pp.tile([128, T], BF16, tag="rms")
            call = pp.tile([128, ET * CLEN], BF16, tag="call")
            hkeep = pp.tile([128, ET * CLEN], BF16, tag="hkeep")
            gkeep = pp.tile([128, ET * CLEN], BF16, tag="gkeep")
            ones = pp.tile([128, 128], BF16, tag="ones")
            psb = pp.tile([128, ET * T], BF16, tag="psb")

            hl_i = dr.tile([128, ET], F32, tag="hli")
            hl_o = dr.tile([128, ET], F32, tag="hlo")

            nc.vector.memset(carry[:], 0.0)
            nc.vector.memset(ones[:], 1.0)

            with tc.tile_pool(name="accp", bufs=1) as ap_, \
                 tc.tile_pool(name="rwk", bufs=1) as rwk, \
                 tc.tile_pool(name="woa", bufs=1) as woap, \
                 tc.tile_pool(name="sp", bufs=1, space="PSUM") as sp:
                acc = ap_.tile([128, T], F32, tag="acc")
                NWA = 4  # leading o_proj weight blocks resident before phase 2
                wo_a = woap.tile([128, NWA * ET * 128], BF16, tag="woa")

                def rms_chunk(tb):
                    # rms[:, tb] = rsqrt(mean(g^2)) = sqrt(D * (1/sum(g^2)))
                    # (EPS=1e-5 vs mean ~0.8 is negligible at bf16 precision;
                    # bf16 partition-reduce keeps the ones-matmul at 1 cyc/row)
                    ts = tb * TQ
                    accb = rwk.tile([128, TQ], BF16, tag="accb")
                    nc.scalar.copy(accb[:], acc[:, ts:ts + TQ])
                    S = sp.tile([128, TQ], F32, tag="S")
                    nc.tensor.matmul(S[:], ones[:], accb[:],
                                     start=True, stop=True)
                    rec = rwk.tile([128, TQ], F32, tag="rec")
                    nc.vector.reciprocal_approx_fast(rec[:], S[:])
                    nc.scalar.activation(rms[:, ts:ts + TQ], rec[:], AF.Sqrt,
                                         scale=float(D))

                # ---------- phase 1: projections + scan, p kept in SBUF -----
                with tc.tile_pool(name="xq", bufs=2) as xqp, \
                     tc.tile_pool(name="wp", bufs=2) as wp, \
                     tc.tile_pool(name="wk", bufs=2) as wk, \
                     tc.tile_pool(name="pj", bufs=2, space="PSUM") as pj:
                    for q in range(NQ):
                        # weights for the first et land before x on q==0 so
                        # the PE starts as early as possible
                        pre_ws = None
                        if q == 0:
                            pre_ws = {}
                            w = wp.tile([128, DT, 128], F8, tag="wf")
                            nc.sync.dma_start(w[:], wf_d.ap()[0:128, :, :])
                            pre_ws["wf"] = w
                        xq = xqp.tile([128, DT * TQ], BF16, tag="xq")
                        xq8 = xqp.tile([128, DT, TQ], F8, tag="xq8")
                        if q == 0:
                            # 8 chunks so the first matmuls can start while
                            # the rest of the quarter streams in (fp8 first:
                            # the f projection leads each et group)
                            for k in range(8):
                                nc.sync.dma_start(
                                    xq8[:, 2 * k:2 * k + 2, :],
                                    xt8_d.ap()[0:128, 2 * k:2 * k + 2, :])
                            for k in range(8):
                                cw = 2 * TQ
                                nc.sync.dma_start(
                                    xq[:, k * cw:(k + 1) * cw],
                                    xt_d.ap()[0:128, k * cw:(k + 1) * cw])
                            for nm, wd in (("wi", wi_d), ("wg", wg_d)):
                                w = wp.tile([128, DT * 128], BF16, tag=nm)
                                nc.sync.dma_start(w[:], wd.ap()[0:128, :])
                                pre_ws[nm] = w
                            nc.sync.dma_start(gnw[:], gnw_d.ap()[:])
                            nc.sync.dma_start(maskt[:], mask_d.ap()[:])
                            nc.sync.dma_start(dfs[:], dfs_d.ap()[:])
                        else:
                            nc.sync.dma_start(
                                xq8[:],
                                xt8_d.ap()[q * 128:(q + 1) * 128, :, :])
                            nc.sync.dma_start(
                                xq[:], xt_d.ap()[q * 128:(q + 1) * 128, :])
                        if q == NQ - 1:
                            # prefetch the leading o_proj weights so phase 2
                            # starts with zero weight-wait
                            for dt in range(NWA):
                                nc.sync.dma_start(
                                    wo_a[:, dt * ET * 128:(dt + 1) * ET * 128],
                                    wo_d.ap()[dt * 128:(dt + 1) * 128, :])
                        ts0 = q * TQ
                        nc.vector.memset(acc[:, ts0:ts0 + TQ], 0.0)
                        for et in range(ET):
                            if q == 0 and et == 0:
                                ws = pre_ws
                            else:
                                ws = {}
                                w = wp.tile([128, DT, 128], F8, tag="wf")
                                nc.sync.dma_start(
                                    w[:],
                                    wf_d.ap()[et * 128:(et + 1) * 128, :, :])
                                ws["wf"] = w
                                for nm, wd in (("wi", wi_d), ("wg", wg_d)):
                                    w = wp.tile([128, DT * 128], BF16, tag=nm)
                                    nc.sync.dma_start(
                                        w[:],
                                        wd.ap()[et * 128:(et + 1) * 128, :])
                                    ws[nm] = w
                            ps = {}
                            p = pj.tile([128, TQ], F32, tag="pf")
                            wf8 = ws["wf"]
                            for u in range(DT // 2):
                                nc.tensor.matmul(
                                    p[:], wf8[:, 2 * u:2 * u + 2, :],
                                    xq8[:, 2 * u:2 * u + 2, :],
                                    start=(u == 0), stop=(u == DT // 2 - 1),
                                    perf_mode=DR)
                            ps["pf"] = p
                            for nm in ("pi", "pg"):
                                p = pj.tile([128, TQ], F32, tag=nm)
                                w = ws["w" + nm[1]]
                                for dt in range(DT):
                                    nc.tensor.matmul(
                                        p[:], w[:, dt * 128:(dt + 1) * 128],
                                        xq[:, dt * TQ:(dt + 1) * TQ],
                                        start=(dt == 0), stop=(dt == DT - 1))
                                ps[nm] = p
                            if et == 0 and q >= 1:
                                # acc for quarter q-1 finalized during our
                                # et=0 matmuls; reduce it now (PE in-order:
                                # emitting later avoids stalling this quarter)
                                rms_chunk(q - 1)
                            gate = wk.tile([128, TQ], BF16, tag="gate")
                            nc.scalar.activation(gate[:], ps["pf"][:],
                                                 AF.Sigmoid,
                                                 scale=dfs[:, 0:1])
                            sil = wk.tile([128, TQ], BF16, tag="sil")
                            nc.scalar.activation(sil[:], ps["pi"][:],
                                                 AF.Sigmoid)
                            nc.vector.tensor_mul(sil[:], ps["pi"][:], sil[:])
                            sq = wk.tile([128, TQ], BF16, tag="sq")
                            nc.scalar.activation(sq[:], ps["pg"][:], AF.Square)
                            omg = wk.tile([128, TQ], BF16, tag="omg")
                            nc.vector.tensor_scalar(omg[:], gate[:], -1.0, 1.0,
                                                    OP.mult, OP.add)
                            iin = wk.tile([128, TQ], BF16, tag="iin")
                            nc.vector.tensor_mul(iin[:], omg[:], sil[:])
                            h1 = wk.tile([128, TQ], F32, tag="h1")
                            nc.vector.tensor_tensor_scan(
                                h1[:], gate[:], iin[:], carry[:, et:et + 1],
                                OP.mult, OP.add)
                            nc.vector.tensor_copy(carry[:, et:et + 1],
                                                  h1[:, TQ - 1:TQ])
                            if q == 0:
                                nc.vector.tensor_tensor_scan(
                                    call[:, et * CLEN:(et + 1) * CLEN],
                                    gate[:, 0:CLEN], gate[:, 0:CLEN], 1.0,
                                    OP.mult, OP.bypass)
                                nc.scalar.copy(
                                    hkeep[:, et * CLEN:(et + 1) * CLEN],
                                    h1[:, 0:CLEN])
                                nc.scalar.copy(
                                    gkeep[:, et * CLEN:(et + 1) * CLEN],
                                    ps["pg"][:, 0:CLEN])
                            nc.vector.tensor_add(acc[:, ts0:ts0 + TQ],
                                                 acc[:, ts0:ts0 + TQ], sq[:])
                            sw = wk.tile([128, TQ], BF16, tag="sw")
                            nc.scalar.activation(sw[:], h1[:], AF.Sigmoid)
                            nc.vector.tensor_mul(sw[:], h1[:], sw[:])
                            nc.vector.scalar_tensor_tensor(
                                psb[:, et * T + ts0:et * T + ts0 + TQ],
                                ps["pg"][:], gnw[:, et:et + 1], sw[:],
                                OP.mult, OP.mult)

                # ---------- phase 1.5: carry exchange + last rms chunk ------
                nc.sync.dma_start(hl_i[:], carry[:])
                nc.gpsimd.collective_compute(
                    "AllReduce", OP.add,
                    replica_groups=[[0, 1], [2, 3], [4, 5], [6, 7]],
                    ins=[hl_i.opt()], outs=[hl_o.opt()])
                rms_chunk(3)

                # ---------- phase 2: gating + output projection -----------------
                with tc.tile_pool(name="wop", bufs=1) as wop, \
                     tc.tile_pool(name="osp", bufs=2) as osp, \
                     tc.tile_pool(name="fxp", bufs=1) as fxp, \
                     tc.tile_pool(name="yp", bufs=4, space="PSUM") as yp, \
                     tc.tile_pool(name="yo", bufs=4) as yo:
                    wo = wop.tile([128, (DT - NWA) * ET * 128], BF16,
                                  tag="wo")
                    for dt in range(NWA, DT):
                        nc.sync.dma_start(
                            wo[:, (dt - NWA) * ET * 128:
                               (dt - NWA + 1) * ET * 128],
                            wo_d.ap()[dt * 128:(dt + 1) * 128, :])
                    for idx, tb2 in enumerate((1, 2, 3, 0)):
                        ts = tb2 * TQ
                        if tb2 == 0:
                            # collective finished ~150us ago; fetch it now so
                            # this DMA never head-of-line blocks the queue
                            nc.sync.dma_start(recv[:], hl_o[:])
                            nc.vector.tensor_sub(recv[:], recv[:], carry[:])
                            nc.vector.tensor_scalar(cin[:], recv[:],
                                                    maskt[:, 0:1], None,
                                                    OP.mult)
                            for et in range(ET):
                                cs = slice(et * CLEN, (et + 1) * CLEN)
                                hfx = fxp.tile([128, CLEN], F32, tag="hfx")
                                nc.vector.scalar_tensor_tensor(
                                    hfx[:], call[:, cs], cin[:, et:et + 1],
                                    hkeep[:, cs], OP.mult, OP.add)
                                swf = fxp.tile([128, CLEN], F32, tag="swf")
                                nc.scalar.activation(swf[:], hfx[:],
                                                     AF.Sigmoid)
                                nc.vector.tensor_mul(swf[:], hfx[:], swf[:])
                                nc.vector.scalar_tensor_tensor(
                                    psb[:, et * T:et * T + CLEN],
                                    gkeep[:, cs], gnw[:, et:et + 1], swf[:],
                                    OP.mult, OP.mult)
                        ot = osp.tile([128, ET * TQ], BF16, tag="osb")
                        for et in range(ET):
                            nc.vector.tensor_mul(
                                ot[:, et * TQ:(et + 1) * TQ],
                                psb[:, et * T + ts:et * T + ts + TQ],
                                rms[:, ts:ts + TQ])
                        for dt in range(DT):
                            if dt < NWA:
                                wsrc, base = wo_a, dt * ET
                            else:
                                wsrc, base = wo, (dt - NWA) * ET
                            ypt = yp.tile([128, TQ], F32, tag="ypt")
                            for et in range(ET):
                                nc.tensor.matmul(
                                    ypt[:],
                                    wsrc[:, (base + et) * 128:
                                       (base + et + 1) * 128],
                                    ot[:, et * TQ:(et + 1) * TQ],
                                    start=(et == 0), stop=(et == ET - 1))
                            ysb = yo.tile([128, TQ], BF16, tag="ysb")
                            nc.scalar.copy(ysb[:], ypt[:])
                            nc.sync.dma_start(
                                yt_d.ap()[dt * 128:(dt + 1) * 128,
                                          ts:ts + TQ], ysb[:])
    nc.compile()
    return nc


def _get_nc():
    if "nc" not in _CACHE:
        _CACHE["nc"] = _build()
    return _CACHE["nc"]


def _q8(a, scale):
    return np.clip(np.asarray(a, np.float32) * scale,
                   -240.0, 240.0).astype(ml_dtypes.float8_e4m3)


def _amax_scale(a, target=224.0):
    return float(target / max(np.abs(np.asarray(a, np.float32)).max(), 1e-30))


def _make_in_maps(hidden_states, Wi, Wf, Wg, g_norm_weight, Wo, **_unused):
    bf = ml_dtypes.bfloat16
    hs = np.asarray(hidden_states, np.float32)
    sx = _amax_scale(hs)
    swf = _amax_scale(Wf)

    def prep_ifg(W):
        # SBUF tile for output block et: w[p, dt*128+e] = W.T[dt*128+p, et*128+e]
        WT = np.ascontiguousarray(np.asarray(W, np.float32).T)
        A = (WT.reshape(DT, 128, ET, 128).transpose(2, 1, 0, 3)
             .reshape(ET * 128, DT * 128))
        return np.ascontiguousarray(A.astype(bf))

    def prep_o(W):
        # SBUF wo[p, (dt*ET+et)*128+d] = W.T[et*128+p, dt*128+d]
        WT = np.ascontiguousarray(np.asarray(W, np.float32).T)
        C = (WT.reshape(ET, 128, DT, 128).transpose(2, 1, 0, 3)
             .reshape(DT * 128, ET * 128))
        return np.ascontiguousarray(C.astype(bf))

    wi = prep_ifg(Wi)
    # f-projection weights in fp8, same layout viewed [ET*128, DT, 128]
    WT = np.ascontiguousarray(np.asarray(Wf, np.float32).T)
    wf = np.ascontiguousarray(_q8(
        (WT.reshape(DT, 128, ET, 128).transpose(2, 1, 0, 3)
         .reshape(ET * 128, DT, 128)), swf))
    wg = prep_ifg(Wg)
    wo = prep_o(Wo)
    gnw = np.ascontiguousarray(
        np.asarray(g_norm_weight, np.float32).reshape(ET, 128).T)
    dfs = np.full((128, 1), 1.0 / (sx * swf), np.float32)
    in_maps = []
    for c in range(NCORE):
        b, half = c // 2, c % 2
        # quarter-major pack: xt[q*128+p, dt*TQ+t] = x[b, half*T + q*TQ+t,
        # dt*128+p] so each quarter is one contiguous [128, DT*TQ] DMA
        blk = hs[b, half * T:(half + 1) * T, :]
        xq4 = blk.reshape(NQ, TQ, DT, 128).transpose(0, 3, 2, 1)
        xt = np.ascontiguousarray(
            xq4.reshape(NQ * 128, DT * TQ)).astype(bf)
        xt8 = np.ascontiguousarray(
            _q8(xq4, sx).reshape(NQ * 128, DT, TQ))
        mask = np.full((128, 1), float(half), np.float32)
        in_maps.append({"xt": xt, "xt8": xt8, "wi": wi, "wf": wf, "wg": wg,
                        "wo": wo, "gnw": gnw, "mask": mask, "dfs": dfs})
    return in_maps


def kernel(hidden_states, Wi, Wf, Wg, g_norm_weight, Wo, **_unused):
    nc = _get_nc()
    in_maps = _make_in_maps(hidden_states, Wi, Wf, Wg, g_norm_weight, Wo)
    _CACHE["in_maps"] = in_maps
    res = run_bass_kernel_spmd(nc, in_maps, list(range(NCORE))).results
    y = np.empty((B, L, D), np.float32)
    for c in range(NCORE):
        b, half = c // 2, c % 2
        y[b, half * T:(half + 1) * T, :] = res[c]["yt"].T.astype(np.float32)
    return y

